# revision 17
# baseline (speedup 1.0000x reference)
"""Trainium2 Bass kernel for nn_BasicLayer (ball sparse-attention + pooling).

Contract: kernel(**inputs) takes the FULL unsharded inputs
(x [65536,256] f32, pos [65536,3] f32, tree_idx_rot [65536] int, params pytree)
and returns the FULL output [32768, 512] f32.

Strategy: shard the 512 balls across 8 NeuronCores (64 balls / core).
Everything runs in ONE SPMD launch:
  block0 (per-core, ball-local) -> AllGather (bf16) -> indirect row-gather
  (applies tree_idx_rot) -> block1 -> AllGather -> row-gather (inverse perm)
  -> pool matmul + BN stats -> AllReduce of stats -> normalize -> output.
Activations feature-major on chip (feat on partitions) so no transposes are
needed inside a block; matmuls run in bf16 (fp32 matmul is 4x slower on PE).
"""

import sys
from contextlib import ExitStack

sys.path.insert(0, "/opt/trn_rl_repo")

import numpy as np
import ml_dtypes

import concourse.bass as bass
import concourse.tile as tile
from concourse import bacc, mybir
from concourse.bass_utils import run_bass_kernel_spmd
from concourse.masks import make_identity
from concourse.tile_rust import add_dep_helper
import concourse.bacc as _bacc_mod
import concourse.hw_specs as _hw_specs

# Prefer the table set containing BOTH Ln and Exp so rmsnorm's
# exp(-0.5*ln(x)) and attention's exp share one resident ACT table set.
_orig_gat = _hw_specs.get_activation_tables

def _gat_pref(arch):
    # act_func_set_id is positional: keep dict order/size intact, but stop
    # exp_and_others / natural_log from being picked so Ln+Exp both resolve
    # to natural_log_exp_and_others (one resident table set, no thrash).
    t = dict(_orig_gat(arch))
    if "natural_log_exp_and_others" in t:
        for k in ("exp_and_others", "natural_log"):
            if k in t:
                t[k] = type(t[k])()
    return t

_bacc_mod.get_activation_tables = _gat_pref

F32 = mybir.dt.float32
BF16 = mybir.dt.bfloat16
I32 = mybir.dt.int32
AF = mybir.ActivationFunctionType
OP = mybir.AluOpType
BF = ml_dtypes.bfloat16

NCORES = 8
DIM = 256
H = 8
E = 32
M = 128
HID = 1024
D3 = 3
RMS_EPS = 1e-6
BN_EPS = 1e-5
GRP = 4  # balls per inner group (free dim 512)

_cache = {}


def _build(NB, single_core=False, only=None):
    """Build + finalize the per-core Bass module for NB balls per core."""
    SH = NB * M               # nodes per core
    NG = NB // GRP            # groups
    NWORLD = 1 if single_core else NCORES
    NTOT = SH * NWORLD        # global nodes
    PR = SH // 2              # pool rows per core
    JG = max(PR // 512, 1)    # pool column groups of <=512
    PJ = PR // JG             # pool rows per group

    def do(name):
        return only is None or name in only

    nc = bacc.Bacc("TRN2", target_bir_lowering=False, debug=False,
                   num_devices=NWORLD)

    _act_last = [None]
    _act_barrier = [None]

    def ACT(*a, **k):
        inst = nc.scalar.activation(*a, **k)
        if _act_barrier[0] is not None:
            add_dep_helper(inst.ins, _act_barrier[0].ins, sync=False,
                           reason="act-table-phase")
        _act_last[0] = inst
        return inst

    def act_boundary():
        if _act_last[0] is not None:
            _act_barrier[0] = _act_last[0]

    def inp(name, shape, dt):
        return nc.dram_tensor(name, shape, dt, kind="ExternalInput")

    # ---- inputs ----
    xT_in = inp("xT", [DIM, SH], F32)
    geoU = [inp(f"geoU{b}", [8, SH], BF16) for b in range(2)]
    geoV = [inp(f"geoV{b}", [8, SH], BF16) for b in range(2)]
    relT_in = [inp(f"relT{b}", [4, SH], BF16) for b in range(2)]
    gidx_in = inp("gidx", [M, NB], I32)
    pidx_in = inp("pidx", [M, NB], I32)
    relp_in = inp("relp", [8, PR], BF16)

    wqk_in = [inp(f"wqk{b}", [M, 2 * 512], BF16) for b in range(2)]
    wv_in = [inp(f"wv{b}", [M, 2 * 256], BF16) for b in range(2)]
    pe_in = [inp(f"pe{b}", [4, 256], BF16) for b in range(2)]
    qkb_in = [inp(f"qkb{b}", [M, 4], F32) for b in range(2)]
    vb_in = [inp(f"vb{b}", [1, 256], BF16) for b in range(2)]
    peb_in = [inp(f"peb{b}", [M, 2], F32) for b in range(2)]
    n1_in = [inp(f"n1_{b}", [M, 2], F32) for b in range(2)]
    n2_in = [inp(f"n2_{b}", [M, 2], F32) for b in range(2)]
    sig_in = [inp(f"sig{b}", [1, H], F32) for b in range(2)]
    proj_in = [inp(f"proj{b}", [M, 2 * 256], BF16) for b in range(2)]
    projb_in = [inp(f"projb{b}", [M, 2], F32) for b in range(2)]
    w12_in = [inp(f"w12_{b}", [M, 2 * 2048], BF16) for b in range(2)]
    w3_in = [inp(f"w3_{b}", [M, 8 * 256], BF16) for b in range(2)]
    w3b_in = [inp(f"w3b{b}", [M, 2], F32) for b in range(2)]
    w12b_in = [inp(f"w12b{b}", [M, 16], F32) for b in range(2)]
    pwab_in = inp("pwab", [M, 2 * 2 * 512], BF16)
    pwr_in = inp("pwr", [8, 512], BF16)
    pbn_in = inp("pbn", [M, 8], F32)

    yout = nc.dram_tensor("yout", [PR, 512], F32, kind="ExternalOutput")

    # ---- internal DRAM (collectives / gather tables) ----
    ag0_in = nc.dram_tensor("ag0_in", [SH, DIM], BF16)
    ag0_out = nc.dram_tensor("ag0_out", [NTOT, DIM], BF16)
    ag1_in = nc.dram_tensor("ag1_in", [SH, DIM], BF16)
    ag1_out = nc.dram_tensor("ag1_out", [NTOT, DIM], BF16)
    st_in = nc.dram_tensor("st_in", [M, 8], F32)
    st_out = nc.dram_tensor("st_out", [M, 8], F32)

    RG = [list(range(NWORLD))]

    with tile.TileContext(nc) as tc:
        with tc.tile_pool(name="const", bufs=1) as cp:
            ident = cp.tile([M, M], BF16, tag="ident")
            make_identity(nc, ident[:])
            ones_bf = cp.tile([M, M], BF16, tag="ones")
            nc.gpsimd.memset(ones_bf[:], 1.0)

            bstack = ExitStack()
            bp = bstack.enter_context(tc.tile_pool(name="blkres", bufs=1))
            xT = [bp.tile([M, SH], F32, tag=f"xT{c}") for c in range(2)]
            Dall = bp.tile([M, SH], BF16, tag="Dall")
            rstd2 = bp.tile([M, SH], BF16, tag="rstd2")
            relT = bp.tile([4, SH], BF16, tag="relT")
            gidx = cp.tile([M, NB], I32, tag="gidx")
            pidx = cp.tile([M, NB], I32, tag="pidx")
            nc.sync.dma_start(gidx[:], gidx_in.ap())
            nc.sync.dma_start(pidx[:], pidx_in.ap())

            # per-block weights (reloaded for block 1)
            wqk = cp.tile([M, 2, 512], BF16, tag="wqk")
            wv = cp.tile([M, 2, 256], BF16, tag="wv")
            pe = cp.tile([4, 256], BF16, tag="pe")
            qkb = cp.tile([M, 4], F32, tag="qkb")
            vb = cp.tile([1, 256], BF16, tag="vb")
            vbb = cp.tile([M, 256], BF16, tag="vbb")
            peb = cp.tile([M, 2], F32, tag="peb")
            n1w = cp.tile([M, 2], F32, tag="n1w")
            n2w = cp.tile([M, 2], F32, tag="n2w")
            sig = cp.tile([1, H], F32, tag="sig")
            sigb = cp.tile([M, H], F32, tag="sigb")
            sigI = cp.tile([M, H * M], BF16, tag="sigI")
            proj = cp.tile([M, 2, 256], BF16, tag="proj")
            projb = cp.tile([M, 2], F32, tag="projb")
            w12 = cp.tile([M, 2, 2048], BF16, tag="w12")
            w3 = cp.tile([M, 8, 256], BF16, tag="w3")
            w3b = cp.tile([M, 2], F32, tag="w3b")
            w12b = cp.tile([M, 16], F32, tag="w12b")

            def load_block_weights(b):
                nc.sync.dma_start(wqk[:], wqk_in[b].ap().rearrange(
                    "p (i o) -> p i o", i=2))
                nc.sync.dma_start(wv[:], wv_in[b].ap().rearrange(
                    "p (i o) -> p i o", i=2))
                nc.sync.dma_start(pe[:], pe_in[b].ap())
                nc.sync.dma_start(qkb[:], qkb_in[b].ap())
                nc.sync.dma_start(vb[:], vb_in[b].ap())
                nc.sync.dma_start(peb[:], peb_in[b].ap())
                nc.sync.dma_start(n1w[:], n1_in[b].ap())
                nc.sync.dma_start(n2w[:], n2_in[b].ap())
                nc.sync.dma_start(sig[:], sig_in[b].ap())
                nc.sync.dma_start(proj[:], proj_in[b].ap().rearrange(
                    "p (i o) -> p i o", i=2))
                nc.sync.dma_start(projb[:], projb_in[b].ap())
                nc.sync.dma_start(w12[:], w12_in[b].ap().rearrange(
                    "p (i o) -> p i o", i=2))
                nc.sync.dma_start(w3[:], w3_in[b].ap().rearrange(
                    "p (h o) -> p h o", h=8))
                nc.sync.dma_start(w3b[:], w3b_in[b].ap())
                nc.sync.dma_start(w12b[:], w12b_in[b].ap())
                nc.sync.dma_start(relT[:], relT_in[b].ap())
                nc.gpsimd.partition_broadcast(sigb[:], sig[:])
                nc.gpsimd.partition_broadcast(vbb[:], vb[:])
                for h in range(H):
                    nc.vector.tensor_scalar_mul(
                        sigI[:, h * M:(h + 1) * M], ident[:],
                        sigb[:, h:h + 1])

            def d_phase(b):
                # distance matrices for all balls -> Dall (bf16)
                with tc.tile_pool(name="dps", bufs=4, space="PSUM") as dps, \
                     tc.tile_pool(name="dwk", bufs=2) as dwk:
                    for g in range(NG):
                        gs = slice(g * GRP * M, (g + 1) * GRP * M)
                        d2 = dps.tile([M, GRP * M], F32, tag="d2")
                        for bi in range(GRP):
                            bs = slice((g * GRP + bi) * M,
                                       (g * GRP + bi + 1) * M)
                            nc.tensor.matmul(
                                d2[:, bi * M:(bi + 1) * M],
                                lhsT=geoU_sb[0:5, bs], rhs=geoV_sb[0:5, bs],
                                start=True, stop=True)
                        dr = dwk.tile([M, GRP * M], F32, tag="dr")
                        ACT(dr[:], d2[:], AF.Relu)
                        ACT(Dall[:, gs], dr[:], AF.Sqrt,
                                             bias=c_dd[:])

            def attn_phase(b):
                with tc.tile_pool(name="aps", bufs=2, space="PSUM") as aps, \
                     tc.tile_pool(name="sps", bufs=1, space="PSUM") as sps, \
                     tc.tile_pool(name="tps", bufs=1, space="PSUM") as tps, \
                     tc.tile_pool(name="ops", bufs=1, space="PSUM") as ops, \
                     tc.tile_pool(name="awk", bufs=2) as awk:
                    for g in range(NG):
                        gs = slice(g * GRP * M, (g + 1) * GRP * M)
                        attn_sb = awk.tile([M, 2, GRP * M], BF16,
                                           tag="attn_sb")
                        # rmsnorm 1
                        x2 = awk.tile([M, GRP * M], BF16, tag="x2")
                        ss = aps.tile([M, GRP * M], F32, tag="ss")
                        for c in range(2):
                            ACT(x2[:], xT[c][:, gs],
                                                 AF.Square)
                            nc.tensor.matmul(ss[:], lhsT=ones_bf[:],
                                             rhs=x2[:], start=(c == 0),
                                             stop=(c == 1))
                        lnv = awk.tile([M, GRP * M], F32, tag="lnv")
                        ACT(lnv[:], ss[:], AF.Ln,
                                             scale=1.0 / DIM, bias=c_rms[:])
                        rstd = awk.tile([M, GRP * M], F32, tag="rstd")
                        ACT(rstd[:], lnv[:], AF.Exp,
                                             scale=-0.5)
                        # x_in = rms(x)*n1 + rel @ pe_w.T + pe_b
                        xin = [awk.tile([M, GRP * M], BF16, tag=f"xin{c}")
                               for c in range(2)]
                        for c in range(2):
                            pe_ps = aps.tile([M, GRP * M], F32, tag="pe_ps")
                            nc.tensor.matmul(
                                pe_ps[:], lhsT=pe[0:3, c * M:(c + 1) * M],
                                rhs=relT[0:3, gs], start=True, stop=True)
                            tmp = awk.tile([M, GRP * M], F32, tag="tmp")
                            nc.vector.scalar_tensor_tensor(
                                tmp[:], in0=xT[c][:, gs],
                                scalar=n1w[:, c:c + 1], in1=rstd[:],
                                op0=OP.mult, op1=OP.mult)
                            nc.vector.scalar_tensor_tensor(
                                xin[c][:], in0=pe_ps[:],
                                scalar=peb[:, c:c + 1], in1=tmp[:],
                                op0=OP.add, op1=OP.add)
                        # q,k (feature-major)
                        qk = awk.tile([M, 4, GRP * M], BF16, tag="qk")
                        for o in range(4):
                            qk_ps = aps.tile([M, GRP * M], F32, tag="qk_ps")
                            for c in range(2):
                                nc.tensor.matmul(
                                    qk_ps[:],
                                    lhsT=wqk[:, c, o * M:(o + 1) * M],
                                    rhs=xin[c][:], start=(c == 0),
                                    stop=(c == 1))
                            ACT(qk[:, o, :], qk_ps[:],
                                                 AF.Identity,
                                                 bias=qkb[:, o:o + 1])
                        for bi in range(GRP):
                            bs = slice((g * GRP + bi) * M,
                                       (g * GRP + bi + 1) * M)
                            bl = slice(bi * M, (bi + 1) * M)
                            # v (node-major)
                            v_ps = aps.tile([M, 256], F32, tag="v_ps")
                            for c in range(2):
                                nc.tensor.matmul(
                                    v_ps[:], lhsT=xin[c][:, bl],
                                    rhs=wv[:, c, :], start=(c == 0),
                                    stop=(c == 1))
                            v_sb = awk.tile([M, 256], BF16, tag="v_sb")
                            nc.vector.scalar_tensor_tensor(
                                v_sb[:], in0=v_ps[:], scalar=1.0,
                                in1=vbb[:], op0=OP.mult, op1=OP.add)
                            # scores: sigma_h * D  (+)  q . k
                            s_ps = sps.tile([M, H * M], F32, tag="s_ps")
                            for h in range(H):
                                nc.tensor.matmul(
                                    s_ps[:, h * M:(h + 1) * M],
                                    lhsT=sigI[:, h * M:(h + 1) * M],
                                    rhs=Dall[:, bs], start=True, stop=False)
                            for h in range(H):
                                hh = h % 4
                                c = h // 4
                                nc.tensor.matmul(
                                    s_ps[:, h * M:(h + 1) * M],
                                    lhsT=qk[hh * E:(hh + 1) * E, c, bl],
                                    rhs=qk[hh * E:(hh + 1) * E, 2 + c, bl],
                                    start=False, stop=True,
                                    tile_position=(hh * E, 0))
                            # softmax (no max subtraction; scores are O(1))
                            a_sb = awk.tile([M, H * M], BF16, tag="a_sb")
                            dsum = awk.tile([M, H], F32, tag="dsum")
                            for h in range(H):
                                ACT(
                                    a_sb[:, h * M:(h + 1) * M],
                                    s_ps[:, h * M:(h + 1) * M], AF.Exp,
                                    accum_out=dsum[:, h:h + 1])
                            dinv = awk.tile([M, H], F32, tag="dinv")
                            nc.vector.reciprocal(dinv[:], dsum[:])
                            for h in range(H):
                                nc.vector.tensor_scalar_mul(
                                    a_sb[:, h * M:(h + 1) * M],
                                    a_sb[:, h * M:(h + 1) * M],
                                    dinv[:, h:h + 1])
                            # transpose A per head (PE), then AV
                            at_ps = tps.tile([M, H * M], BF16, tag="at_ps")
                            for h in range(H):
                                nc.tensor.transpose(
                                    at_ps[:, h * M:(h + 1) * M],
                                    a_sb[:, h * M:(h + 1) * M], ident[:])
                            at_sb = awk.tile([M, H * M], BF16, tag="at_sb")
                            nc.vector.tensor_copy(at_sb[:], at_ps[:])
                            o_ps = ops.tile([M, 256], F32, tag="o_ps")
                            for half in range(2):
                                for j in range(4):
                                    h = half * 4 + j
                                    nc.tensor.matmul(
                                        o_ps[j * E:(j + 1) * E,
                                             half * M:(half + 1) * M],
                                        lhsT=v_sb[:, h * E:(h + 1) * E],
                                        rhs=at_sb[:, h * M:(h + 1) * M],
                                        start=True, stop=True,
                                        tile_position=(0, j * E),
                                        skip_group_check=True)
                            ACT(
                                attn_sb[:, 0, bl], o_ps[:, 0:M], AF.Copy)
                            ACT(
                                attn_sb[:, 1, bl], o_ps[:, M:256], AF.Copy)
                        # proj + residual
                        for oc in range(2):
                            p_ps = aps.tile([M, GRP * M], F32, tag="p_ps")
                            for ic in range(2):
                                nc.tensor.matmul(
                                    p_ps[:],
                                    lhsT=proj[:, ic, oc * M:(oc + 1) * M],
                                    rhs=attn_sb[:, ic, :], start=(ic == 0),
                                    stop=(ic == 1))
                            nc.vector.scalar_tensor_tensor(
                                xT[oc][:, gs], in0=p_ps[:],
                                scalar=projb[:, oc:oc + 1],
                                in1=xT[oc][:, gs], op0=OP.add, op1=OP.add)

            def rms2_phase(b):
                with tc.tile_pool(name="rps", bufs=4, space="PSUM") as rps, \
                     tc.tile_pool(name="rwk", bufs=2) as rwk:
                    for g in range(NG):
                        gs = slice(g * GRP * M, (g + 1) * GRP * M)
                        x2 = rwk.tile([M, GRP * M], BF16, tag="x2")
                        ss = rps.tile([M, GRP * M], F32, tag="ss")
                        for c in range(2):
                            ACT(x2[:], xT[c][:, gs],
                                                 AF.Square)
                            nc.tensor.matmul(ss[:], lhsT=ones_bf[:],
                                             rhs=x2[:], start=(c == 0),
                                             stop=(c == 1))
                        lnv = rwk.tile([M, GRP * M], F32, tag="lnv")
                        ACT(lnv[:], ss[:], AF.Ln,
                                             scale=1.0 / DIM, bias=c_rms[:])
                        ACT(rstd2[:, gs], lnv[:], AF.Exp,
                                             scale=-0.5)

            def mlp_phase(b):
                with tc.tile_pool(name="mps", bufs=4, space="PSUM") as mps, \
                     tc.tile_pool(name="yps", bufs=1, space="PSUM") as yps, \
                     tc.tile_pool(name="mwk", bufs=3) as mwk:
                    for g in range(NG):
                        gs = slice(g * GRP * M, (g + 1) * GRP * M)
                        xn = [mwk.tile([M, GRP * M], BF16, tag=f"xn{c}")
                              for c in range(2)]
                        for c in range(2):
                            nc.vector.scalar_tensor_tensor(
                                xn[c][:], in0=xT[c][:, gs],
                                scalar=n2w[:, c:c + 1], in1=rstd2[:, gs],
                                op0=OP.mult, op1=OP.mult)
                        y3 = [yps.tile([M, GRP * M], F32, tag=f"y3_{oc}")
                              for oc in range(2)]
                        for hc in range(8):
                            h1 = mps.tile([M, GRP * M], F32, tag="h1")
                            h2 = mps.tile([M, GRP * M], F32, tag="h2")
                            for c in range(2):
                                nc.tensor.matmul(
                                    h1[:],
                                    lhsT=w12[:, c, hc * M:(hc + 1) * M],
                                    rhs=xn[c][:], start=(c == 0),
                                    stop=(c == 1))
                            for c in range(2):
                                nc.tensor.matmul(
                                    h2[:],
                                    lhsT=w12[:, c, 1024 + hc * M:
                                             1024 + (hc + 1) * M],
                                    rhs=xn[c][:], start=(c == 0),
                                    stop=(c == 1))
                            h1s = mwk.tile([M, GRP * M], BF16, tag="h1s")
                            ACT(h1s[:], h1[:], AF.Silu,
                                                 bias=w12b[:, hc:hc + 1])
                            gt = mwk.tile([M, GRP * M], BF16, tag="gt")
                            nc.vector.scalar_tensor_tensor(
                                gt[:], in0=h2[:],
                                scalar=w12b[:, 8 + hc:9 + hc],
                                in1=h1s[:], op0=OP.add, op1=OP.mult)
                            for oc in range(2):
                                nc.tensor.matmul(
                                    y3[oc][:],
                                    lhsT=w3[:, hc, oc * M:(oc + 1) * M],
                                    rhs=gt[:], start=(hc == 0),
                                    stop=(hc == 7))
                        for oc in range(2):
                            nc.vector.scalar_tensor_tensor(
                                xT[oc][:, gs], in0=y3[oc][:],
                                scalar=w3b[:, oc:oc + 1],
                                in1=xT[oc][:, gs], op0=OP.add, op1=OP.add)

            def write_node_major(ag_dst):
                # xT (f32, feature-major) -> bf16 node-major rows in DRAM
                with tc.tile_pool(name="wps", bufs=4, space="PSUM") as wps, \
                     tc.tile_pool(name="wwk", bufs=3) as wwk:
                    for g in range(NG):
                        gs = slice(g * GRP * M, (g + 1) * GRP * M)
                        xc = [wwk.tile([M, GRP * M], BF16, tag=f"xc{c}")
                              for c in range(2)]
                        for c in range(2):
                            nc.vector.tensor_copy(xc[c][:], xT[c][:, gs])
                        t_ps = wps.tile([M, GRP * 2 * M], BF16, tag="t_ps")
                        for bi in range(GRP):
                            for c in range(2):
                                nc.tensor.transpose(
                                    t_ps[:, (bi * 2 + c) * M:
                                         (bi * 2 + c + 1) * M],
                                    xc[c][:, bi * M:(bi + 1) * M], ident[:])
                        y0 = wwk.tile([M, GRP * 2 * M], BF16, tag="y0")
                        nc.vector.tensor_copy(y0[:], t_ps[:])
                        dst = ag_dst.ap()[g * GRP * M:(g + 1) * GRP * M, :] \
                            .rearrange("(bi p) f -> p bi f", p=M)
                        nc.sync.dma_start(dst, y0[:].rearrange(
                            "p (bi f) -> p bi f", bi=GRP))

            def gather_feature_major(ag_src, idx_t, dst_tiles, dst_dt):
                # rows idx from DRAM -> feature-major dst (2 chunk tiles)
                with tc.tile_pool(name="gps", bufs=4, space="PSUM") as gps, \
                     tc.tile_pool(name="gwk", bufs=3) as gwk:
                    for g in range(NB):
                        xg = gwk.tile([M, DIM], BF16, tag="xg")
                        nc.gpsimd.indirect_dma_start(
                            out=xg[:], out_offset=None, in_=ag_src.ap(),
                            in_offset=bass.IndirectOffsetOnAxis(
                                ap=idx_t[:, g:g + 1], axis=0))
                        t_ps = gps.tile([M, DIM], BF16, tag="t_ps")
                        for c in range(2):
                            nc.tensor.transpose(
                                t_ps[:, c * M:(c + 1) * M],
                                xg[:, c * M:(c + 1) * M], ident[:])
                        for c in range(2):
                            nc.vector.tensor_copy(
                                dst_tiles[c][:, g * M:(g + 1) * M],
                                t_ps[:, c * M:(c + 1) * M])

            # ================= block 0 =================
            nc.sync.dma_start(xT[0][:], xT_in.ap()[0:M, :])
            nc.sync.dma_start(xT[1][:], xT_in.ap()[M:DIM, :])
            load_block_weights(0)
            with tc.tile_pool(name="geo", bufs=1) as gp:
                geoU_sb = gp.tile([8, SH], BF16, tag="geoU")
                geoV_sb = gp.tile([8, SH], BF16, tag="geoV")
                nc.sync.dma_start(geoU_sb[:], geoU[0].ap())
                nc.sync.dma_start(geoV_sb[:], geoV[0].ap())
                if do("d"):
                    d_phase(0)
            if do("attn"):
                attn_phase(0)
            if do("rms2"):
                rms2_phase(0)
            act_boundary()
            if do("mlp"):
                mlp_phase(0)
            act_boundary()
            if do("wnm"):
                write_node_major(ag0_in)
            if single_core:
                nc.sync.dma_start(ag0_out.ap(), ag0_in.ap())
            else:
                nc.gpsimd.collective_compute(
                    "AllGather", OP.bypass, replica_groups=RG,
                    ins=[ag0_in.ap().opt()], outs=[ag0_out.ap().opt()])

            # ================= block 1 =================
            if do("gather"):
                gather_feature_major(ag0_out, gidx, xT, F32)
            load_block_weights(1)
            with tc.tile_pool(name="geo", bufs=1) as gp:
                geoU_sb = gp.tile([8, SH], BF16, tag="geoU")
                geoV_sb = gp.tile([8, SH], BF16, tag="geoV")
                nc.sync.dma_start(geoU_sb[:], geoU[1].ap())
                nc.sync.dma_start(geoV_sb[:], geoV[1].ap())
                if do("d"):
                    d_phase(1)
            if do("attn"):
                attn_phase(1)
            if do("rms2"):
                rms2_phase(1)
            act_boundary()
            if do("mlp"):
                mlp_phase(1)
            act_boundary()
            if do("wnm"):
                write_node_major(ag1_in)
            if single_core:
                nc.sync.dma_start(ag1_out.ap(), ag1_in.ap())
            else:
                nc.gpsimd.collective_compute(
                    "AllGather", OP.bypass, replica_groups=RG,
                    ins=[ag1_in.ap().opt()], outs=[ag1_out.ap().opt()])

            # ================= pooling =================
            bstack.close()
            y1T = [cp.tile([M, SH], BF16, tag=f"y1T{c}") for c in range(2)]
            gather_feature_major(ag1_out, pidx, y1T, BF16)

            pwab = cp.tile([M, 2, 2, 512], BF16, tag="pwab")
            nc.sync.dma_start(pwab[:], pwab_in.ap().rearrange(
                "p (i a o) -> p i a o", i=2, a=2))
            pwr = cp.tile([8, 512], BF16, tag="pwr")
            nc.sync.dma_start(pwr[:], pwr_in.ap())
            pbn = cp.tile([M, 8], F32, tag="pbn")
            nc.sync.dma_start(pbn[:], pbn_in.ap())
            relp_sb = cp.tile([8, PR], BF16, tag="relp")
            nc.sync.dma_start(relp_sb[:], relp_in.ap())

            yP = [cp.tile([M, PR], F32, tag=f"yP{oc}") for oc in range(4)]
            sacc = cp.tile([M, 4, JG], F32, tag="sacc")
            qacc = cp.tile([M, 4, JG], F32, tag="qacc")

            with tc.tile_pool(name="pps", bufs=4, space="PSUM") as pps, \
                 tc.tile_pool(name="pwk", bufs=2) as pwk:
                for jg in range(JG):
                    js = slice(jg * PJ, (jg + 1) * PJ)
                    for oc in range(4):
                        ps = pps.tile([M, PJ], F32, tag="ps")
                        first = True
                        for ab in range(2):
                            src = [t[:, 2 * jg * PJ:2 * (jg + 1) * PJ]
                                   .rearrange("p (j two) -> p two j", two=2)
                                   [:, ab, :] for t in y1T]
                            for ic in range(2):
                                nc.tensor.matmul(
                                    ps[:],
                                    lhsT=pwab[:, ic, ab,
                                              oc * M:(oc + 1) * M],
                                    rhs=src[ic], start=first, stop=False)
                                first = False
                        nc.tensor.matmul(
                            ps[:], lhsT=pwr[0:6, oc * M:(oc + 1) * M],
                            rhs=relp_sb[0:6, js], start=False, stop=True)
                        ACT(yP[oc][:, js], ps[:],
                                             AF.Identity,
                                             accum_out=sacc[:, oc, jg:jg + 1])
                        sqd = pwk.tile([M, PJ], BF16, tag="sqd")
                        ACT(sqd[:], ps[:], AF.Square,
                                             accum_out=qacc[:, oc, jg:jg + 1])

            # global BN stats via AllReduce
            st_sb = cp.tile([M, 8], F32, tag="st_sb")
            nc.vector.tensor_reduce(st_sb[:, 0:4], sacc[:],
                                    axis=mybir.AxisListType.X, op=OP.add)
            nc.vector.tensor_reduce(st_sb[:, 4:8], qacc[:],
                                    axis=mybir.AxisListType.X, op=OP.add)
            nc.sync.dma_start(st_in.ap(), st_sb[:])
            nc.gpsimd.collective_compute(
                "AllReduce", OP.add, replica_groups=RG,
                ins=[st_in.ap().opt()], outs=[st_out.ap().opt()])
            st = cp.tile([M, 8], F32, tag="st")
            nc.sync.dma_start(st[:], st_out.ap())

            NROWS = float(PR * NWORLD)
            mean = cp.tile([M, 4], F32, tag="mean")
            nc.vector.tensor_scalar_mul(mean[:], st[:, 0:4], 1.0 / NROWS)
            ex2 = cp.tile([M, 4], F32, tag="ex2")
            nc.vector.tensor_scalar_mul(ex2[:], st[:, 4:8], 1.0 / NROWS)
            m2 = cp.tile([M, 4], F32, tag="m2")
            nc.vector.tensor_tensor(m2[:], in0=mean[:], in1=mean[:],
                                    op=OP.mult)
            var = cp.tile([M, 4], F32, tag="var")
            nc.vector.tensor_tensor(var[:], in0=ex2[:], in1=m2[:],
                                    op=OP.subtract)
            sd = cp.tile([M, 4], F32, tag="sd")
            ACT(sd[:], var[:], AF.Sqrt, bias=c_bn[:])
            rin = cp.tile([M, 4], F32, tag="rin")
            nc.vector.reciprocal(rin[:], sd[:])
            sc = cp.tile([M, 4], F32, tag="sc")
            nc.vector.tensor_tensor(sc[:], in0=pbn[:, 0:4], in1=rin[:],
                                    op=OP.mult)
            ms = cp.tile([M, 4], F32, tag="ms")
            nc.vector.tensor_tensor(ms[:], in0=mean[:], in1=sc[:],
                                    op=OP.mult)
            tt = cp.tile([M, 4], F32, tag="tt")
            nc.vector.tensor_tensor(tt[:], in0=pbn[:, 4:8], in1=ms[:],
                                    op=OP.subtract)

            # normalize, transpose to node-major, write out
            with tc.tile_pool(name="fps", bufs=4, space="PSUM") as fps, \
                 tc.tile_pool(name="fwk", bufs=3) as fwk:
                ysc = [cp.tile([M, PR], BF16, tag=f"ysc{oc}")
                       for oc in range(4)]
                for oc in range(4):
                    ACT(ysc[oc][:], yP[oc][:], AF.Identity,
                                         scale=sc[:, oc:oc + 1],
                                         bias=tt[:, oc:oc + 1])
                for jt in range(PR // M):
                    t_ps = fps.tile([M, 512], BF16, tag="t_ps")
                    for oc in range(4):
                        nc.tensor.transpose(
                            t_ps[:, oc * M:(oc + 1) * M],
                            ysc[oc][:, jt * M:(jt + 1) * M], ident[:])
                    yo = fwk.tile([M, 512], F32, tag="yo")
                    nc.vector.tensor_copy(yo[:], t_ps[:])
                    nc.sync.dma_start(
                        yout.ap()[jt * M:(jt + 1) * M, :], yo[:])

    nc.finalize()
    return nc


def _prep_inputs(x, pos, tree_idx_rot, params, NB):
    """Host-side sharding + weight packing. Returns in_maps for 8 cores."""
    SH = NB * M
    NTOT = SH * NCORES
    PR = SH // 2
    idx = np.asarray(tree_idx_rot).astype(np.int64)
    inv = np.argsort(idx)
    x = np.asarray(x, dtype=np.float32)
    pos = np.asarray(pos, dtype=np.float32)

    xT = np.ascontiguousarray(x.T)  # [256, N]

    def geo_arrays(p):
        # p: [N, 3] in the block's node order
        pb = p.reshape(-1, M, D3)
        centers = pb.mean(axis=1)
        rel = (pb - centers[:, None]).reshape(-1, D3).T  # [3, N]
        sq = (p * p).sum(axis=1)  # [N]
        U = np.zeros((8, p.shape[0]), np.float32)
        U[0:3] = p.T
        U[3] = sq
        U[4] = 1.0
        V = np.zeros((8, p.shape[0]), np.float32)
        V[0:3] = -2.0 * p.T
        V[3] = 1.0
        V[4] = sq
        R = np.zeros((4, p.shape[0]), np.float32)
        R[0:3] = rel
        return U.astype(BF), V.astype(BF), R.astype(BF)

    U0, V0, R0 = geo_arrays(pos)
    pos_p = pos[idx]
    U1, V1, R1 = geo_arrays(pos_p)

    # pool rel features (original node order)
    pb = pos.reshape(-1, 2, D3)
    pc = pb.mean(axis=1)
    prel = (pb - pc[:, None]).reshape(-1, 2 * D3).T  # [6, N/2]
    relp = np.zeros((8, NTOT // 2), np.float32)
    relp[0:6] = prel
    relp = relp.astype(BF)

    scale = 1.0 / np.sqrt(float(E))

    def block_weights(p):
        qkv_w = np.asarray(p["qkv_w"], np.float32)   # [768, 256]
        qkv_b = np.asarray(p["qkv_b"], np.float32)
        # channel c = h*96 + e*3 + k
        ordq = np.array([h * 96 + e * 3 + 0 for h in range(H)
                         for e in range(E)])
        Wq = qkv_w[ordq] * scale
        Wk = qkv_w[ordq + 1]
        Wv = qkv_w[ordq + 2]
        bq = qkv_b[ordq] * scale
        bk = qkv_b[ordq + 1]
        bv = qkv_b[ordq + 2]
        WqT, WkT, WvT = Wq.T, Wk.T, Wv.T  # [in 256, out 256]
        wqk = np.zeros((M, 2, 512), np.float32)
        for ic in range(2):
            wqk[:, ic, 0:256] = WqT[ic * M:(ic + 1) * M]
            wqk[:, ic, 256:512] = WkT[ic * M:(ic + 1) * M]
        wv = np.zeros((M, 2, 256), np.float32)
        for ic in range(2):
            wv[:, ic, :] = WvT[ic * M:(ic + 1) * M]
        qkb = np.stack([bq[0:M], bq[M:256], bk[0:M], bk[M:256]], axis=1)
        pe_w = np.asarray(p["pe_w"], np.float32)  # [256, 3]
        pe = np.zeros((4, 256), np.float32)
        pe[0:3] = pe_w.T
        peb = np.asarray(p["pe_b"], np.float32).reshape(2, M).T  # [128,2]
        n1 = np.asarray(p["norm1"], np.float32).reshape(2, M).T
        n2 = np.asarray(p["norm2"], np.float32).reshape(2, M).T
        sig = np.asarray(p["sigma"], np.float32).reshape(1, H)
        proj_w = np.asarray(p["proj_w"], np.float32)
        projT = proj_w.T
        proj = np.zeros((M, 2, 256), np.float32)
        for ic in range(2):
            proj[:, ic, :] = projT[ic * M:(ic + 1) * M]
        projb = np.asarray(p["proj_b"], np.float32).reshape(2, M).T
        w1T = np.asarray(p["w1_w"], np.float32).T  # [256, 1024]
        w2T = np.asarray(p["w2_w"], np.float32).T
        w12 = np.zeros((M, 2, 2048), np.float32)
        for ic in range(2):
            w12[:, ic, 0:1024] = w1T[ic * M:(ic + 1) * M]
            w12[:, ic, 1024:2048] = w2T[ic * M:(ic + 1) * M]
        w12b = np.zeros((M, 16), np.float32)
        w12b[:, 0:8] = np.asarray(p["w1_b"], np.float32).reshape(8, M).T
        w12b[:, 8:16] = np.asarray(p["w2_b"], np.float32).reshape(8, M).T
        w3T = np.asarray(p["w3_w"], np.float32).T  # [1024, 256]
        w3 = np.zeros((M, 8, 256), np.float32)
        for hc in range(8):
            w3[:, hc, :] = w3T[hc * M:(hc + 1) * M]
        w3b = np.asarray(p["w3_b"], np.float32).reshape(2, M).T
        return dict(
            wqk=wqk.reshape(M, -1).astype(BF), wv=wv.reshape(M, -1).astype(BF),
            pe=pe.astype(BF), qkb=qkb.astype(np.float32),
            vb=bv.reshape(1, 256).astype(BF), peb=peb.astype(np.float32),
            n1=n1.astype(np.float32), n2=n2.astype(np.float32),
            sig=sig.astype(np.float32),
            proj=proj.reshape(M, -1).astype(BF),
            projb=projb.astype(np.float32),
            w12=w12.reshape(M, -1).astype(BF),
            w3=w3.reshape(M, -1).astype(BF), w3b=w3b.astype(np.float32),
            w12b=w12b.astype(np.float32))

    bw = [block_weights(params["blocks"][b]) for b in range(2)]

    pool_w = np.asarray(params["pool_w"], np.float32)  # [512, 518]
    WaT = pool_w[:, 0:256].T    # [256, 512]
    WbT = pool_w[:, 256:512].T
    WrT = pool_w[:, 512:518].T  # [6, 512]
    pwab = np.zeros((M, 2, 2, 512), np.float32)
    for ic in range(2):
        pwab[:, ic, 0, :] = WaT[ic * M:(ic + 1) * M]
        pwab[:, ic, 1, :] = WbT[ic * M:(ic + 1) * M]
    pwr = np.zeros((8, 512), np.float32)
    pwr[0:6] = WrT
    bn_g = np.asarray(params["bn_g"], np.float32)
    bn_b = np.asarray(params["bn_b"], np.float32)
    pool_b = np.asarray(params["pool_b"], np.float32)
    assert np.all(pool_b == 0) or True  # pool_b cancels inside BN
    pbn = np.zeros((M, 8), np.float32)
    pbn[:, 0:4] = bn_g.reshape(4, M).T
    pbn[:, 4:8] = bn_b.reshape(4, M).T

    in_maps = []
    for c in range(NCORES):
        ns = slice(c * SH, (c + 1) * SH)
        js = slice(c * PR, (c + 1) * PR)
        m = {
            "xT": np.ascontiguousarray(xT[:, ns]),
            "geoU0": np.ascontiguousarray(U0[:, ns]),
            "geoV0": np.ascontiguousarray(V0[:, ns]),
            "relT0": np.ascontiguousarray(R0[:, ns]),
            "geoU1": np.ascontiguousarray(U1[:, ns]),
            "geoV1": np.ascontiguousarray(V1[:, ns]),
            "relT1": np.ascontiguousarray(R1[:, ns]),
            "gidx": np.ascontiguousarray(
                idx[ns].reshape(NB, M).T.astype(np.int32)),
            "pidx": np.ascontiguousarray(
                inv[ns].reshape(NB, M).T.astype(np.int32)),
            "relp": np.ascontiguousarray(relp[:, js]),
            "pwab": pwab.reshape(M, -1).astype(BF),
            "pwr": pwr.astype(BF),
            "pbn": pbn,
        }
        for b in range(2):
            w = bw[b]
            m.update({
                f"wqk{b}": w["wqk"], f"wv{b}": w["wv"], f"pe{b}": w["pe"],
                f"qkb{b}": w["qkb"], f"vb{b}": w["vb"], f"peb{b}": w["peb"],
                f"n1_{b}": w["n1"], f"n2_{b}": w["n2"], f"sig{b}": w["sig"],
                f"proj{b}": w["proj"], f"projb{b}": w["projb"],
                f"w12_{b}": w["w12"], f"w3_{b}": w["w3"], f"w3b{b}": w["w3b"],
                f"w12b{b}": w["w12b"],
            })
        in_maps.append(m)
    return in_maps


def run(x, pos, tree_idx_rot, params, NB=64, trace=False):
    if NB not in _cache:
        _cache[NB] = _build(NB)
    nc = _cache[NB]
    in_maps = _prep_inputs(x, pos, tree_idx_rot, params, NB)
    res = run_bass_kernel_spmd(nc, in_maps, core_ids=list(range(NCORES)),
                               trace=trace)
    out = np.concatenate([res.results[c]["yout"] for c in range(NCORES)],
                         axis=0)
    return out, res


def kernel(x, pos, tree_idx_rot, params):
    out, _ = run(x, pos, tree_idx_rot, params, NB=64)
    return out.astype(np.float32)
